# revision 1
# baseline (speedup 1.0000x reference)
"""Trainium2 Bass kernel for the CustomCRFLoss problem.

Strategy (pure data parallel, one sample per NeuronCore, 8 cores):

The reference computes, per sample:
    unary  = softplus(d) - label*d            (d = l1 - l0, 2 classes)
    val[i,j,w'] = exp(-di/2) + exp(-dj/2)     (128^3 pairwise Gaussian kernels)
    5 x mean-field:  Q <- Q - (P@Q)           (P@Q)[i,j] = sum_w val[i,j,w]*colsum(Q)[w]
    out = mean(Q)

Key reduction: the output only needs column sums.  With
    M[j,w] = sum_i val[i,j,w]   (a single 128x128 matrix per sample)
the 5 iterations collapse to 5 matvecs:
    q_{t+1} = q_t - M q_t,  answer = (sum(unary) - sum_t 1^T M q_t) / (n*h*w)

M = M1 + M2:
  * spatial part M1[j,w] = sum_i exp(-||x_ij - x_iw||^2/2) is computed with a
    degree-5 Taylor feature map phi_m(x) = x^alpha/sqrt(alpha!) * exp(-r/2)
    (x centered, so |<x,x'>| <= 0.75 and the truncation error is ~2e-4):
    M1 = sum_m T_m^T T_m  -- 56 bf16 matmuls accumulated in PSUM, zero exps.
  * bilateral part M2[j,w] = sum_i exp(-||x_ij - x_wj||^2/2) via 128 small
    K=5 bf16 Gram matmuls (augmented with -r/2 rows so PSUM holds the full
    exponent), batched ACT exp, and DVE row-sum reduction (the per-column
    kernel matrix is symmetric, so row sums equal the needed column sums).

Emulated accuracy vs the JAX reference: rel err ~2.5e-5.
"""

import math

import numpy as np

import concourse.bass as bass
import concourse.tile as tile
from concourse import mybir
from concourse.bass_utils import run_bass_kernel_spmd
from concourse.tile import add_dep_helper

H = W = 128
PIX = H * W
NB = 8  # batch / cores
DEG = 4

F32 = mybir.dt.float32
BF16 = mybir.dt.bfloat16
AF = mybir.ActivationFunctionType
ALU = mybir.AluOpType
AX = mybir.AxisListType

# bilateral grouping: GRP j-columns per PSUM group tile
GRP = 8
NGRP = W // GRP


def _monomial_ops(deg):
    """Canonically ordered monomials of degree<=deg in 3 vars + build schedule.

    Order within degree k: [x1^k] + x2*(c0-prefix of L(k-1)) + x3*L(k-1).
    This makes every op's parents AND children contiguous ranges, and the
    x3-children's scale (1/sqrt(c+1)) constant on runs of equal parent c.
    Returns (total_count, ops) with ops = (child_off, parent_off, width,
    channel, scale); each feature tile ends up as x^alpha/sqrt(alpha!)*E0.
    """
    L = [[(0, 0, 0)]]
    for k in range(1, deg + 1):
        prev = L[-1]
        cur = [(prev[0][0] + 1, prev[0][1], prev[0][2])]
        cur += [(a, b + 1, 0) for (a, b, c) in prev if c == 0]
        cur += [(a, b, c + 1) for (a, b, c) in prev]
        L.append(cur)
    offs = [0]
    for lst in L:
        offs.append(offs[-1] + len(lst))
    ops = []
    for k in range(1, deg + 1):
        po, co = offs[k - 1], offs[k]
        prev = L[k - 1]
        ops.append((co, po, 1, 0, k))
        for t in range(k):
            ops.append((co + 1 + t, po + t, 1, 1, t + 1))
        base = co + 1 + k
        i = 0
        while i < len(prev):
            cval = prev[i][2]
            jx = i
            while jx < len(prev) and prev[jx][2] == cval:
                jx += 1
            ops.append((base + i, po + i, jx - i, 2, cval + 1))
            i = jx
    return offs[-1], ops


NMON, MONOPS = _monomial_ops(DEG)


def _bcast(ap, wid):
    """[P,128] AP -> [P,wid,128] with a step-0 middle dim."""
    return bass.AP(
        tensor=ap.tensor,
        offset=ap.offset,
        ap=[list(ap.ap[0]), [0, wid], list(ap.ap[1])],
    )


def build_kernel():
    nc = bass.Bass()
    lg_d = nc.dram_tensor("logits", (2, H, W), F32, kind="ExternalInput")
    lb_d = nc.dram_tensor("labels", (H, W), F32, kind="ExternalInput")
    im_d = nc.dram_tensor("imb", (3, H, W), BF16, kind="ExternalInput")
    imT_d = nc.dram_tensor("imtb", (3, W, H), BF16, kind="ExternalInput")
    out_d = nc.dram_tensor("out", (1, H), F32, kind="ExternalOutput")

    with tile.TileContext(nc) as tc:
        with (
            tc.tile_pool(name="sb", bufs=1) as sb,
            tc.tile_pool(name="ex", bufs=3) as expp,
            tc.tile_pool(name="qp", bufs=3) as qpool,
            tc.tile_pool(name="pg", bufs=2, space="PSUM") as pg,
            tc.tile_pool(name="pm", bufs=1, space="PSUM") as pm,
            tc.tile_pool(name="pm2", bufs=1, space="PSUM") as pm2,
            tc.tile_pool(name="ps", bufs=1, space="PSUM") as ps,
        ):
            # -------- inputs: bf16 pre-centered images, both layouts --------
            # J row order: JL = [ones, nrT, x1T, x2T, x3T]
            #              JR = [nrT, ones, x1T, x2T, x3T]
            # pairs: 1*nrT (free side) + nrT*1 (partition side) + x*x = exponent
            JL = sb.tile([5, PIX], BF16)
            JR = sb.tile([5, PIX], BF16)
            ttile = sb.tile([W, 3, H], BF16)
            nc.sync.dma_start(out=ttile, in_=imT_d[:].rearrange("c j i -> j c i"))
            imtile = sb.tile([H, 3, W], BF16)
            nc.scalar.dma_start(out=imtile, in_=im_d[:].rearrange("c i j -> i c j"))
            nc.sync.dma_start(
                out=JL[2:5, :], in_=imT_d[:].rearrange("c j i -> c (j i)")
            )
            nc.scalar.dma_start(
                out=JR[2:5, :], in_=imT_d[:].rearrange("c j i -> c (j i)")
            )
            xb = [imtile[:, c, :] for c in range(3)]
            tbs = [ttile[:, c, :] for c in range(3)]
            # ---------------- constants ----------------
            ones_col = sb.tile([H, 1], F32)
            nc.vector.memset(ones_col, 1.0)
            ones_b = sb.tile([H, W], BF16)
            nc.vector.memset(ones_b, 1.0)
            nc.gpsimd.dma_start(out=JL[0:1, :], in_=ones_b)
            nc.gpsimd.dma_start(out=JR[1:2, :], in_=ones_b)
            ones_cb = sb.tile([H, 1], BF16)
            nc.vector.memset(ones_cb, 1.0)

            # ---------------- nrT (gates bilateral), then nr ----------------
            def _sumsq(srcs, tag):
                s1 = sb.tile([H, W], F32, tag=f"{tag}1")
                nc.vector.tensor_mul(out=s1, in0=srcs[0], in1=srcs[0])
                s2 = sb.tile([H, W], F32, tag=f"{tag}2")
                nc.vector.tensor_mul(out=s2, in0=srcs[1], in1=srcs[1])
                s12 = sb.tile([H, W], F32, tag=f"{tag}3")
                nc.vector.tensor_add(out=s12, in0=s1, in1=s2)
                s3 = sb.tile([H, W], F32, tag=f"{tag}4")
                nc.vector.tensor_mul(out=s3, in0=srcs[2], in1=srcs[2])
                o = sb.tile([H, W], F32, tag=f"{tag}5")
                nc.vector.tensor_add(out=o, in0=s12, in1=s3)
                return o

            rT = _sumsq(tbs, "rT")
            nrT_b = sb.tile([W, H], BF16)
            nc.vector.tensor_scalar_mul(out=nrT_b, in0=rT, scalar1=-0.5)
            nc.sync.dma_start(out=JL[1:2, :], in_=nrT_b)
            nc.gpsimd.dma_start(out=JR[0:1, :], in_=nrT_b)
            lg = sb.tile([H, 2, W], F32)
            nc.sync.dma_start(out=lg, in_=lg_d[:].rearrange("c i j -> i c j"))
            lb = sb.tile([H, W], F32)
            nc.sync.dma_start(out=lb, in_=lb_d[:])
            rr = _sumsq(xb, "rr")
            nr = sb.tile([H, W], F32)
            nc.vector.tensor_scalar_mul(out=nr, in0=rr, scalar1=-0.5)

            # ---------------- bilateral: G -> exp -> rowsum ----------------
            # mt2[w', j] = sum_i E_j[i, w']  (E_j symmetric -> row sums)
            mt2 = pm2.tile([H, W], F32)
            gmm_first = {}
            for g in range(NGRP):
                gp = pg.tile([H, GRP, W], F32, tag="g")
                for t in range(GRP):
                    j = g * GRP + t
                    _mm = nc.tensor.matmul(
                        gp[:, t, :],
                        lhsT=JL[:, j * W : (j + 1) * W],
                        rhs=JR[:, j * W : (j + 1) * W],
                        start=True,
                        stop=True,
                    )
                    if t == 0:
                        gmm_first[g] = _mm.ins
                ech = expp.tile([H, GRP, W], BF16, tag="ech")
                nc.scalar.activation(out=ech, in_=gp, func=AF.Exp)
                for t in range(GRP):
                    j = g * GRP + t
                    nc.tensor.matmul(
                        mt2[:, j : j + 1],
                        lhsT=ech[:, t, :],
                        rhs=ones_cb,
                        start=True,
                        stop=True,
                    )

            # ---------------- unary ----------------
            dd = sb.tile([H, W], F32)
            nc.vector.tensor_sub(out=dd, in0=lg[:, 1, :], in1=lg[:, 0, :])
            ed = sb.tile([H, W], F32)
            nc.scalar.activation(out=ed, in_=dd, func=AF.Exp)
            sp = sb.tile([H, W], F32)
            nc.scalar.activation(out=sp, in_=ed, func=AF.Ln, bias=1.0)
            tl = sb.tile([H, W], F32)
            nc.vector.tensor_mul(out=tl, in0=lb, in1=dd)
            u = sb.tile([H, W], F32)
            nc.vector.tensor_sub(out=u, in0=sp, in1=tl)

            # ---------------- spatial features (tile-major) ----------------
            # pre-scaled multipliers xs[c][e] = xb[c]/sqrt(e) (tensor_scalar: 4x mode)
            xs = {}
            for c in range(3):
                xs[(c, 1)] = xb[c]
                for e in range(2, DEG + 1):
                    t = sb.tile([H, W], BF16, tag=f"xs{c}_{e}")
                    nc.vector.tensor_scalar_mul(
                        out=t, in0=xb[c], scalar1=1.0 / math.sqrt(e)
                    )
                    xs[(c, e)] = t

            TT = sb.tile([H, NMON, W], BF16)
            nc.scalar.activation(out=TT[:, 0, :], in_=nr, func=AF.Exp)
            for (co, po, wid, ch, e) in MONOPS:
                mult = xs[(ch, e)]
                nc.vector.tensor_mul(
                    out=TT[:, co : co + wid, :],
                    in0=TT[:, po : po + wid, :],
                    in1=_bcast(mult[:], wid) if wid > 1 else mult[:],
                )

            mt1 = pm.tile([H, W], F32)
            for m in range(NMON):
                _mm = nc.tensor.matmul(
                    mt1,
                    lhsT=TT[:, m, :],
                    rhs=TT[:, m, :],
                    start=(m == 0),
                    stop=(m == NMON - 1),
                )
                # keep PE free for the bilateral pipe early on: slot the
                # spatial matmuls into PE gaps of the last bilateral groups
                anchor = NGRP - 4 + min(3, m * 4 // NMON)
                add_dep_helper(
                    _mm.ins, gmm_first[anchor], False, "interleave after bilateral"
                )

            # ---------------- M, q0, iterations ----------------
            # AT = I - M^T; q_{t+1} = q_t - M q_t; answer = 1^T q_5
            mt2s = sb.tile([H, W], F32)
            nc.vector.tensor_copy(out=mt2s, in_=mt2)
            MT = sb.tile([H, W], F32)
            nc.vector.tensor_add(out=MT, in0=mt1, in1=mt2s)

            q0p = ps.tile([H, 1], F32, tag="s")
            nc.tensor.matmul(q0p, lhsT=u, rhs=ones_col, start=True, stop=True)
            qcur = qpool.tile([H, 1], F32, tag="q")
            nc.vector.tensor_copy(out=qcur, in_=q0p)

            for it in range(5):
                yp = ps.tile([H, 1], F32, tag="s")
                nc.tensor.matmul(yp, lhsT=MT, rhs=qcur, start=True, stop=True)
                qn = qpool.tile([H, 1], F32, tag="q")
                nc.vector.tensor_sub(out=qn, in0=qcur, in1=yp)
                qcur = qn

            nc.sync.dma_start(out=out_d[:], in_=qcur)

    return nc


def _split_excess_waits(nc, max_waits=1, max_updates=1):
    """The walrus build in this container rejects instructions whose Events
    carry more than one semaphore wait (ISA Events has a single wait slot).
    Tile's sem assignment can attach several.  Split the extras onto
    same-engine NoOps placed immediately before (waits) / after (updates)
    the instruction; sequencers execute in order, so semantics are kept."""
    for fn in nc.m.functions:
        for bb in fn.blocks:
            ins = bb.instructions
            out = []
            changed = False
            for inst in ins:
                si = inst.sync_info
                if si is None:
                    out.append(inst)
                    continue
                waits = list(si.on_wait or [])
                updates = list(si.on_update or [])
                if len(waits) <= max_waits and len(updates) <= max_updates:
                    out.append(inst)
                    continue
                changed = True
                pre, post = [], []
                if len(waits) > max_waits:
                    for k, wt in enumerate(waits[:-max_waits]):
                        pre.append(
                            mybir.InstNoOp(
                                name=f"{inst.name}-w{k}",
                                engine=inst.engine,
                                bass_nofuse=True,
                                sync_info=mybir.SyncInfo(on_wait=[wt], on_update=[]),
                            )
                        )
                    waits = waits[-max_waits:]
                if len(updates) > max_updates:
                    for k, up in enumerate(updates[max_updates:]):
                        post.append(
                            mybir.InstNoOp(
                                name=f"{inst.name}-u{k}",
                                engine=inst.engine,
                                bass_nofuse=True,
                                sync_info=mybir.SyncInfo(on_wait=[], on_update=[up]),
                            )
                        )
                    updates = updates[:max_updates]
                inst.sync_info = mybir.SyncInfo(on_wait=waits, on_update=updates)
                out.extend(pre)
                out.append(inst)
                out.extend(post)
            if changed:
                bb.instructions = out
    return nc


_NC_CACHE = None


def kernel(logits, labels, images):
    global _NC_CACHE
    if _NC_CACHE is None:
        _NC_CACHE = _split_excess_waits(build_kernel())
    nc = _NC_CACHE

    import ml_dtypes

    logits = np.ascontiguousarray(np.asarray(logits, dtype=np.float32))
    labels_f = np.ascontiguousarray(np.asarray(labels).astype(np.float32))
    images = np.asarray(images, dtype=np.float32)
    imc = images - 0.5
    im_b = np.ascontiguousarray(imc.astype(ml_dtypes.bfloat16))
    imT_b = np.ascontiguousarray(np.swapaxes(imc, 2, 3).astype(ml_dtypes.bfloat16))

    in_maps = [
        {
            "logits": logits[b],
            "labels": labels_f[b],
            "imb": im_b[b],
            "imtb": imT_b[b],
        }
        for b in range(NB)
    ]
    res = run_bass_kernel_spmd(nc, in_maps, core_ids=list(range(NB)))
    tot = 0.0
    for b in range(NB):
        tot += float(res.results[b]["out"].astype(np.float64).sum())
    return np.float32(tot / (NB * H * W))



# revision 3
# speedup vs baseline: 2.5076x; 2.5076x over previous
"""Trainium2 Bass kernel for the CustomCRFLoss problem.

Strategy (pure data parallel, one sample per NeuronCore, 8 cores):

The reference collapses to (see baseline derivation):
    s_0[j] = colsum(unary)[j],  s_{t+1} = s_t - M s_t,  answer = 1^T s_5
with M[j,w] = M1[j,w] + M2[j,w]:
    M1[j,w] = sum_i k(x_ij, x_iw)   (spatial, row pairs)
    M2[j,w] = sum_i k(x_ij, x_wj)   (bilateral, column pairs)
and k(a,b) = exp(-||a-b||^2/2) = exp(-|a|^2/2) exp(-|b|^2/2) exp(a.b).

Both terms use ONE degree-2 Taylor feature tile TT[p,m,j] = phi_m(x at row
p, col j) (10 monomial features x^alpha/sqrt(alpha!) * exp(-r/2), bf16):
    MT1 = sum_m T_m^T T_m                      (10 PSUM-accumulated matmuls)
    MT2[w,j] = sum_m T_m[w,j] * c_m[j],  c_m[j] = sum_i T_m[i,j]
The partition-broadcast column sums come free from CC = J @ TT with J the
all-ones matrix (3 matmuls of 512 cols), so MT2 is 3 elementwise muls and a
small add tree.  No exps beyond the single E0, no per-column Gram matmuls.

Tail: 4 iterations of (matvec + subtract), then the 5th iteration is folded
into a single dot product: answer = <1 - rowsum(MT), s_4>.

Emulated accuracy vs the f64 reference: rel err ~1.3e-3 (gate 2e-2).
"""

import math

import numpy as np

import concourse.bass as bass
import concourse.tile as tile
from concourse import mybir
from concourse.bass_utils import run_bass_kernel_spmd

H = W = 128
NB = 8  # batch / cores
NMON = 10

F32 = mybir.dt.float32
BF16 = mybir.dt.bfloat16
AF = mybir.ActivationFunctionType
ALU = mybir.AluOpType
AX = mybir.AxisListType

ISQ2 = 1.0 / math.sqrt(2.0)


def _bcast(ap, wid):
    """[P,128] AP -> [P,wid,128] with a step-0 middle dim."""
    return bass.AP(
        tensor=ap.tensor,
        offset=ap.offset,
        ap=[list(ap.ap[0]), [0, wid], list(ap.ap[1])],
    )


def build_kernel():
    nc = bass.Bass()
    im_d = nc.dram_tensor("imb", (H, 3, W), BF16, kind="ExternalInput")
    lg_d = nc.dram_tensor("lg", (H, 2, W), F32, kind="ExternalInput")
    lb_d = nc.dram_tensor("lb", (H, W), F32, kind="ExternalInput")
    out_d = nc.dram_tensor("out", (1, 1), F32, kind="ExternalOutput")

    with tile.TileContext(nc) as tc:
        with (
            tc.tile_pool(name="sb", bufs=1) as sb,
            tc.tile_pool(name="qp", bufs=3) as qpool,
            tc.tile_pool(name="pm", bufs=1, space="PSUM") as pm,
            tc.tile_pool(name="pc", bufs=1, space="PSUM") as pc,
            tc.tile_pool(name="ps", bufs=2, space="PSUM") as ps,
        ):
            # ---------------- input DMAs (dense, partition-major) ----------
            imtile = sb.tile([H, 3, W], BF16)
            nc.sync.dma_start(out=imtile, in_=im_d[:])
            lg = sb.tile([H, 2, W], F32)
            nc.sync.dma_start(out=lg, in_=lg_d[:])
            lb = sb.tile([H, W], F32)
            nc.sync.dma_start(out=lb, in_=lb_d[:])

            # ---------------- constants ----------------
            ones_col = sb.tile([H, 1], F32)
            nc.gpsimd.memset(ones_col, 1.0)
            ones_mat = sb.tile([H, W], BF16)
            nc.gpsimd.memset(ones_mat, 1.0)

            # ---------------- feature build ----------------
            # xsall = x/sqrt(2); sq2 = x*xsall = x^2/sqrt(2)
            xsall = sb.tile([H, 3, W], BF16)
            nc.vector.tensor_scalar_mul(out=xsall, in0=imtile, scalar1=ISQ2)
            sq2 = sb.tile([H, 3, W], BF16)
            nc.vector.tensor_mul(out=sq2, in0=imtile, in1=xsall)
            # r/sqrt(2) = sum_c sq2_c ; E0 = exp(-r/2) via ACT scale
            a1 = sb.tile([H, W], BF16)
            nc.vector.tensor_add(out=a1, in0=sq2[:, 0, :], in1=sq2[:, 1, :])
            rs = sb.tile([H, W], BF16)
            nc.vector.tensor_add(out=rs, in0=a1, in1=sq2[:, 2, :])

            # TT slots: 0=E0, 1:4=x_c*E0, 4:7=x_c^2/sqrt2*E0, 7=x1x2E0,
            # 8=x1x3E0, 9=x2x3E0
            TT = sb.tile([H, NMON, W], BF16)
            nc.scalar.activation(
                out=TT[:, 0, :], in_=rs, func=AF.Exp, scale=-ISQ2
            )
            E0 = TT[:, 0, :]
            nc.vector.tensor_mul(out=TT[:, 1:4, :], in0=imtile, in1=_bcast(E0, 3))
            nc.vector.tensor_mul(out=TT[:, 4:7, :], in0=sq2, in1=_bcast(E0, 3))
            T1 = TT[:, 1, :]
            nc.vector.tensor_mul(
                out=TT[:, 7:9, :], in0=_bcast(T1, 2), in1=imtile[:, 1:3, :]
            )
            nc.vector.tensor_mul(
                out=TT[:, 9:10, :], in0=TT[:, 2:3, :], in1=imtile[:, 2:3, :]
            )

            # ---------------- unary (off critical path) ----------------
            dd = sb.tile([H, W], F32)
            nc.vector.tensor_sub(out=dd, in0=lg[:, 1, :], in1=lg[:, 0, :])
            ed = sb.tile([H, W], F32)
            nc.scalar.activation(out=ed, in_=dd, func=AF.Exp)
            sp = sb.tile([H, W], F32)
            nc.scalar.activation(out=sp, in_=ed, func=AF.Ln, bias=1.0)
            tl = sb.tile([H, W], F32)
            nc.vector.tensor_mul(out=tl, in0=lb, in1=dd)
            u = sb.tile([H, W], F32)
            nc.vector.tensor_sub(out=u, in0=sp, in1=tl)

            # ---------------- PE: MT1 accumulation + CC chunks -------------
            mt1 = pm.tile([H, W], F32)
            ccp = []
            chunks = [(0, 4), (4, 8), (8, 10)]
            for m in range(NMON):
                nc.tensor.matmul(
                    mt1,
                    lhsT=TT[:, m, :],
                    rhs=TT[:, m, :],
                    start=(m == 0),
                    stop=(m == NMON - 1),
                )
                for k, (a, b) in enumerate(chunks):
                    if m == b - 1:
                        cp = pc.tile([H, (b - a) * W], F32, tag=f"cc{k}")
                        nc.tensor.matmul(
                            cp,
                            lhsT=ones_mat,
                            rhs=TT[:, a:b, :],
                            start=True,
                            stop=True,
                        )
                        ccp.append(cp)

            # ---------------- CC -> SBUF bf16 (ACT), products, tree --------
            ccs = sb.tile([H, NMON, W], BF16)
            for k, (a, b) in enumerate(chunks):
                nc.scalar.activation(
                    out=ccs[:, a:b, :], in_=ccp[k], func=AF.Copy
                )
            P = sb.tile([H, NMON, W], BF16)
            for k, (a, b) in enumerate(chunks):
                nc.vector.tensor_mul(
                    out=P[:, a:b, :], in0=TT[:, a:b, :], in1=ccs[:, a:b, :]
                )
            t1 = sb.tile([H, 2, W], BF16)
            nc.vector.tensor_add(out=t1, in0=P[:, 0:2, :], in1=P[:, 2:4, :])
            t2 = sb.tile([H, 2, W], BF16)
            nc.vector.tensor_add(out=t2, in0=P[:, 4:6, :], in1=P[:, 6:8, :])
            t3 = sb.tile([H, W], BF16)
            nc.vector.tensor_add(out=t3, in0=P[:, 8, :], in1=P[:, 9, :])
            u1 = sb.tile([H, 2, W], BF16)
            nc.vector.tensor_add(out=u1, in0=t1, in1=t2)
            u2 = sb.tile([H, W], BF16)
            nc.vector.tensor_add(out=u2, in0=u1[:, 0, :], in1=u1[:, 1, :])
            mt2b = sb.tile([H, W], BF16)
            nc.vector.tensor_add(out=mt2b, in0=u2, in1=t3)

            # ---------------- M, h, iterations ----------------
            MTs = sb.tile([H, W], F32)
            nc.vector.tensor_add(out=MTs, in0=mt1, in1=mt2b)
            g = sb.tile([H, 1], F32)
            nc.vector.tensor_reduce(out=g, in_=MTs, axis=AX.X, op=ALU.add)
            hv = sb.tile([H, 1], F32)
            nc.vector.tensor_scalar(
                out=hv, in0=g, scalar1=-1.0, scalar2=1.0,
                op0=ALU.mult, op1=ALU.add,
            )

            q0p = ps.tile([H, 1], F32, tag="s")
            nc.tensor.matmul(q0p, lhsT=u, rhs=ones_col, start=True, stop=True)
            qcur = qpool.tile([H, 1], F32, tag="q")
            nc.vector.tensor_copy(out=qcur, in_=q0p)

            for it in range(4):
                yp = ps.tile([H, 1], F32, tag="s")
                nc.tensor.matmul(yp, lhsT=MTs, rhs=qcur, start=True, stop=True)
                qn = qpool.tile([H, 1], F32, tag="q")
                nc.vector.tensor_sub(out=qn, in0=qcur, in1=yp)
                qcur = qn

            # answer = <1 - rowsum(MT), s_4> = 1^T s_5
            ansp = ps.tile([1, 1], F32, tag="a")
            nc.tensor.matmul(ansp, lhsT=hv, rhs=qcur, start=True, stop=True)
            ans = qpool.tile([1, 1], F32, tag="ans")
            nc.vector.tensor_copy(out=ans, in_=ansp)
            nc.sync.dma_start(out=out_d[:], in_=ans)

    return nc


def _split_excess_waits(nc, max_waits=1, max_updates=1):
    """The walrus build in this container rejects instructions whose Events
    carry more than one semaphore wait (ISA Events has a single wait slot).
    Tile's sem assignment can attach several.  Split the extras onto
    same-engine NoOps placed immediately before (waits) / after (updates)
    the instruction; sequencers execute in order, so semantics are kept."""
    for fn in nc.m.functions:
        for bb in fn.blocks:
            ins = bb.instructions
            out = []
            changed = False
            for inst in ins:
                si = inst.sync_info
                if si is None:
                    out.append(inst)
                    continue
                waits = list(si.on_wait or [])
                updates = list(si.on_update or [])
                if len(waits) <= max_waits and len(updates) <= max_updates:
                    out.append(inst)
                    continue
                changed = True
                pre, post = [], []
                if len(waits) > max_waits:
                    for k, wt in enumerate(waits[:-max_waits]):
                        pre.append(
                            mybir.InstNoOp(
                                name=f"{inst.name}-w{k}",
                                engine=inst.engine,
                                bass_nofuse=True,
                                sync_info=mybir.SyncInfo(on_wait=[wt], on_update=[]),
                            )
                        )
                    waits = waits[-max_waits:]
                if len(updates) > max_updates:
                    for k, up in enumerate(updates[max_updates:]):
                        post.append(
                            mybir.InstNoOp(
                                name=f"{inst.name}-u{k}",
                                engine=inst.engine,
                                bass_nofuse=True,
                                sync_info=mybir.SyncInfo(on_wait=[], on_update=[up]),
                            )
                        )
                    updates = updates[:max_updates]
                inst.sync_info = mybir.SyncInfo(on_wait=waits, on_update=updates)
                out.extend(pre)
                out.append(inst)
                out.extend(post)
            if changed:
                bb.instructions = out
    return nc


_NC_CACHE = None


def kernel(logits, labels, images):
    global _NC_CACHE
    if _NC_CACHE is None:
        _NC_CACHE = _split_excess_waits(build_kernel())
    nc = _NC_CACHE

    import ml_dtypes

    logits = np.asarray(logits, dtype=np.float32)
    labels_f = np.asarray(labels).astype(np.float32)
    images = np.asarray(images, dtype=np.float32)
    imc = images - 0.5
    # partition-major layouts: [H, C, W]
    im_b = np.ascontiguousarray(np.swapaxes(imc, 1, 2).astype(ml_dtypes.bfloat16))
    lg_t = np.ascontiguousarray(np.swapaxes(logits, 1, 2))

    in_maps = [
        {
            "imb": im_b[b],
            "lg": lg_t[b],
            "lb": np.ascontiguousarray(labels_f[b]),
        }
        for b in range(NB)
    ]
    res = run_bass_kernel_spmd(nc, in_maps, core_ids=list(range(NB)))
    tot = 0.0
    for b in range(NB):
        tot += float(res.results[b]["out"].astype(np.float64).sum())
    return np.float32(tot / (NB * H * W))


# revision 4
# speedup vs baseline: 2.6668x; 1.0635x over previous
"""Trainium2 Bass kernel for the CustomCRFLoss problem.

Strategy (pure data parallel, one sample per NeuronCore, 8 cores):

The reference collapses to:
    s_0[j] = colsum(unary)[j],  s_{t+1} = s_t - M s_t,  answer = 1^T s_5
with M[j,w] = M1[j,w] + M2[j,w]:
    M1[j,w] = sum_i k(x_ij, x_iw)   (spatial, row pairs)
    M2[j,w] = sum_i k(x_ij, x_wj)   (bilateral, column pairs)
and k(a,b) = exp(-||a-b||^2/2) = exp(-|a|^2/2) exp(-|b|^2/2) exp(a.b).

Both terms use ONE degree-2 Taylor feature tile TT[p,m,j] = phi_m(x at row
p, col j) (10 monomial features x^alpha/sqrt(alpha!) * exp(-r/2), bf16):
    MT1 = sum_m T_m^T T_m                      (10 PSUM-accumulated matmuls)
    MT2[w,j] = sum_m T_m[w,j] * c_m[j],  c_m[j] = sum_i T_m[i,j]
The partition-broadcast column sums come free from CC = J @ TT with J the
all-ones matrix (3 matmuls), so MT2 is 3 elementwise muls and a small add
tree.  No exps beyond the single E0, no per-column Gram matmuls.

Slot order is chosen so CC chunk 0 = [E0, diag] depends only on E0 (the
earliest features), and the unary runs on the otherwise idle Pool engine.
Tail: 4 iterations of (matvec + subtract); the 5th iteration is folded into
a host-side dot: answer = <1 - rowsum(MT), s_4>, with rowsum(MT) from the
Activation engine's accumulate output.  The kernel ships [s_4 | g] per core.

Emulated accuracy vs the f64 reference: rel err ~1e-3 (gate 2e-2).
"""

import math

import numpy as np

import concourse.bass as bass
import concourse.tile as tile
from concourse import mybir
from concourse.bass_utils import run_bass_kernel_spmd

H = W = 128
NB = 8  # batch / cores
NMON = 10

F32 = mybir.dt.float32
BF16 = mybir.dt.bfloat16
AF = mybir.ActivationFunctionType
ALU = mybir.AluOpType
AX = mybir.AxisListType

ISQ2 = 1.0 / math.sqrt(2.0)


def _bcast(ap, wid):
    """[P,128] AP -> [P,wid,128] with a step-0 middle dim."""
    return bass.AP(
        tensor=ap.tensor,
        offset=ap.offset,
        ap=[list(ap.ap[0]), [0, wid], list(ap.ap[1])],
    )


def build_kernel():
    nc = bass.Bass()
    im_d = nc.dram_tensor("imb", (H, 3, W), BF16, kind="ExternalInput")
    lg_d = nc.dram_tensor("lg", (H, 2, W), F32, kind="ExternalInput")
    lb_d = nc.dram_tensor("lb", (H, W), F32, kind="ExternalInput")
    out_d = nc.dram_tensor("out", (H, 2), F32, kind="ExternalOutput")

    with tile.TileContext(nc) as tc:
        with (
            tc.tile_pool(name="sb", bufs=1) as sb,
            tc.tile_pool(name="qp", bufs=3) as qpool,
            tc.tile_pool(name="pm", bufs=1, space="PSUM") as pm,
            tc.tile_pool(name="pc", bufs=1, space="PSUM") as pc,
            tc.tile_pool(name="ps", bufs=2, space="PSUM") as ps,
        ):
            # ---------------- input DMAs (dense, partition-major) ----------
            imtile = sb.tile([H, 3, W], BF16)
            nc.sync.dma_start(out=imtile, in_=im_d[:])
            lg = sb.tile([H, 2, W], F32)
            nc.sync.dma_start(out=lg, in_=lg_d[:])
            lb = sb.tile([H, W], F32)
            nc.sync.dma_start(out=lb, in_=lb_d[:])

            # ---------------- constants ----------------
            ones_mat = sb.tile([H, W], BF16)
            nc.gpsimd.memset(ones_mat, 1.0)
            ones_col = sb.tile([H, 1], F32)
            nc.gpsimd.memset(ones_col, 1.0)

            # ------------- feature build (DVE critical path) ---------------
            # xsall = x/sqrt(2); sq2 = x*xsall = x^2/sqrt(2)
            xsall = sb.tile([H, 3, W], BF16)
            nc.vector.tensor_scalar_mul(out=xsall, in0=imtile, scalar1=ISQ2)
            sq2 = sb.tile([H, 3, W], BF16)
            nc.vector.tensor_mul(out=sq2, in0=imtile, in1=xsall)
            # r/sqrt(2) = sum_c sq2_c ; E0 = exp(-r/2) via ACT scale
            a1 = sb.tile([H, W], BF16)
            nc.vector.tensor_add(out=a1, in0=sq2[:, 0, :], in1=sq2[:, 1, :])
            rs = sb.tile([H, W], BF16)
            nc.vector.tensor_add(out=rs, in0=a1, in1=sq2[:, 2, :])

            # TT slots: 0=E0, 1:4=x_c^2/sqrt2*E0 (diag), 4:7=x_c*E0 (deg1),
            # 7=x1x2E0, 8=x1x3E0, 9=x2x3E0
            TT = sb.tile([H, NMON, W], BF16)
            nc.scalar.activation(
                out=TT[:, 0, :], in_=rs, func=AF.Exp, scale=-ISQ2
            )
            E0 = TT[:, 0, :]
            nc.vector.tensor_mul(out=TT[:, 1:4, :], in0=sq2, in1=_bcast(E0, 3))
            nc.vector.tensor_mul(out=TT[:, 4:7, :], in0=imtile, in1=_bcast(E0, 3))
            T1 = TT[:, 4, :]
            nc.vector.tensor_mul(
                out=TT[:, 7:9, :], in0=_bcast(T1, 2), in1=imtile[:, 1:3, :]
            )
            nc.vector.tensor_mul(
                out=TT[:, 9:10, :], in0=TT[:, 5:6, :], in1=imtile[:, 2:3, :]
            )

            # ---------------- unary (Pool + ACT, off critical path) --------
            dd = sb.tile([H, W], F32)
            nc.gpsimd.tensor_sub(out=dd, in0=lg[:, 1, :], in1=lg[:, 0, :])
            ed = sb.tile([H, W], F32)
            nc.scalar.activation(out=ed, in_=dd, func=AF.Exp)
            sp = sb.tile([H, W], F32)
            nc.scalar.activation(out=sp, in_=ed, func=AF.Ln, bias=1.0)
            tl = sb.tile([H, W], F32)
            nc.gpsimd.tensor_mul(out=tl, in0=lb, in1=dd)
            u = sb.tile([H, W], F32)
            nc.gpsimd.tensor_sub(out=u, in0=sp, in1=tl)

            # ---------------- PE: CC chunks first, then MT1 ----------------
            chunks = [(0, 4), (4, 8), (8, 10)]
            mt1 = pm.tile([H, W], F32)
            nc.tensor.matmul(
                mt1, lhsT=TT[:, 0, :], rhs=TT[:, 0, :], start=True, stop=False
            )
            cc0 = pc.tile([H, 4 * W], F32, tag="cc0")
            nc.tensor.matmul(
                cc0, lhsT=ones_mat, rhs=TT[:, 0:4, :], start=True, stop=True
            )
            for m in range(1, 4):
                nc.tensor.matmul(
                    mt1, lhsT=TT[:, m, :], rhs=TT[:, m, :], start=False, stop=False
                )
            cc1 = pc.tile([H, 4 * W], F32, tag="cc1")
            nc.tensor.matmul(
                cc1, lhsT=ones_mat, rhs=TT[:, 4:8, :], start=True, stop=True
            )
            cc2 = pc.tile([H, 2 * W], F32, tag="cc2")
            nc.tensor.matmul(
                cc2, lhsT=ones_mat, rhs=TT[:, 8:10, :], start=True, stop=True
            )
            for m in range(4, NMON):
                nc.tensor.matmul(
                    mt1, lhsT=TT[:, m, :], rhs=TT[:, m, :],
                    start=False, stop=(m == NMON - 1),
                )
            q0p = ps.tile([H, 1], F32, tag="s")
            nc.tensor.matmul(q0p, lhsT=u, rhs=ones_col, start=True, stop=True)

            # -------- products (DVE, PSUM-direct) + chunk-local tree -------
            # last chunk goes through an ACT copy so its product is 2x
            ccs2 = sb.tile([H, 2, W], BF16)
            nc.scalar.activation(out=ccs2, in_=cc2, func=AF.Copy)

            P = sb.tile([H, NMON, W], BF16)
            nc.vector.tensor_mul(out=P[:, 0:4, :], in0=TT[:, 0:4, :], in1=cc0)
            r1a = sb.tile([H, 2, W], BF16)
            nc.vector.tensor_add(out=r1a, in0=P[:, 0:2, :], in1=P[:, 2:4, :])
            r1s = sb.tile([H, W], BF16)
            nc.vector.tensor_add(out=r1s, in0=r1a[:, 0, :], in1=r1a[:, 1, :])
            nc.vector.tensor_mul(out=P[:, 4:8, :], in0=TT[:, 4:8, :], in1=cc1)
            s1a = sb.tile([H, 2, W], BF16)
            nc.vector.tensor_add(out=s1a, in0=P[:, 4:6, :], in1=P[:, 6:8, :])
            s1s = sb.tile([H, W], BF16)
            nc.vector.tensor_add(out=s1s, in0=s1a[:, 0, :], in1=s1a[:, 1, :])
            f1 = sb.tile([H, W], BF16)
            nc.vector.tensor_add(out=f1, in0=r1s, in1=s1s)
            nc.vector.tensor_mul(out=P[:, 8:10, :], in0=TT[:, 8:10, :], in1=ccs2)
            t3 = sb.tile([H, W], BF16)
            nc.vector.tensor_add(out=t3, in0=P[:, 8, :], in1=P[:, 9, :])
            mt2b = sb.tile([H, W], BF16)
            nc.vector.tensor_add(out=mt2b, in0=f1, in1=t3)

            # ---------------- M, iterations ----------------
            qcur = qpool.tile([H, 1], F32, tag="q")
            nc.vector.tensor_copy(out=qcur, in_=q0p)
            MTs = sb.tile([H, W], F32)
            nc.vector.tensor_add(out=MTs, in0=mt1, in1=mt2b)

            # out layout: vout[:,0]=s_4, vout[:,1]=g=rowsum(MT)
            vout = sb.tile([H, 2], F32)
            gdump = sb.tile([H, W], BF16)
            nc.scalar.activation(
                out=gdump, in_=MTs, func=AF.Copy, accum_out=vout[:, 1:2]
            )

            for it in range(4):
                yp = ps.tile([H, 1], F32, tag="s")
                nc.tensor.matmul(yp, lhsT=MTs, rhs=qcur, start=True, stop=True)
                if it < 3:
                    qn = qpool.tile([H, 1], F32, tag="q")
                    nc.vector.tensor_sub(out=qn, in0=qcur, in1=yp)
                    qcur = qn
                else:
                    nc.vector.tensor_sub(out=vout[:, 0:1], in0=qcur, in1=yp)

            nc.sync.dma_start(out=out_d[:], in_=vout)

    return nc


def _split_excess_waits(nc, max_waits=1, max_updates=1):
    """The walrus build in this container rejects instructions whose Events
    carry more than one semaphore wait (ISA Events has a single wait slot).
    Tile's sem assignment can attach several.  Split the extras onto
    same-engine NoOps placed immediately before (waits) / after (updates)
    the instruction; sequencers execute in order, so semantics are kept."""
    for fn in nc.m.functions:
        for bb in fn.blocks:
            ins = bb.instructions
            out = []
            changed = False
            for inst in ins:
                si = inst.sync_info
                if si is None:
                    out.append(inst)
                    continue
                waits = list(si.on_wait or [])
                updates = list(si.on_update or [])
                if len(waits) <= max_waits and len(updates) <= max_updates:
                    out.append(inst)
                    continue
                changed = True
                pre, post = [], []
                if len(waits) > max_waits:
                    for k, wt in enumerate(waits[:-max_waits]):
                        pre.append(
                            mybir.InstNoOp(
                                name=f"{inst.name}-w{k}",
                                engine=inst.engine,
                                bass_nofuse=True,
                                sync_info=mybir.SyncInfo(on_wait=[wt], on_update=[]),
                            )
                        )
                    waits = waits[-max_waits:]
                if len(updates) > max_updates:
                    for k, up in enumerate(updates[max_updates:]):
                        post.append(
                            mybir.InstNoOp(
                                name=f"{inst.name}-u{k}",
                                engine=inst.engine,
                                bass_nofuse=True,
                                sync_info=mybir.SyncInfo(on_wait=[], on_update=[up]),
                            )
                        )
                    updates = updates[:max_updates]
                inst.sync_info = mybir.SyncInfo(on_wait=waits, on_update=updates)
                out.extend(pre)
                out.append(inst)
                out.extend(post)
            if changed:
                bb.instructions = out
    return nc


_NC_CACHE = None


def kernel(logits, labels, images):
    global _NC_CACHE
    if _NC_CACHE is None:
        _NC_CACHE = _split_excess_waits(build_kernel())
    nc = _NC_CACHE

    import ml_dtypes

    logits = np.asarray(logits, dtype=np.float32)
    labels_f = np.asarray(labels).astype(np.float32)
    images = np.asarray(images, dtype=np.float32)
    imc = images - 0.5
    # partition-major layouts: [H, C, W]
    im_b = np.ascontiguousarray(np.swapaxes(imc, 1, 2).astype(ml_dtypes.bfloat16))
    lg_t = np.ascontiguousarray(np.swapaxes(logits, 1, 2))

    in_maps = [
        {
            "imb": im_b[b],
            "lg": lg_t[b],
            "lb": np.ascontiguousarray(labels_f[b]),
        }
        for b in range(NB)
    ]
    res = run_bass_kernel_spmd(nc, in_maps, core_ids=list(range(NB)))
    tot = 0.0
    for b in range(NB):
        o = res.results[b]["out"].astype(np.float64)
        s4, g = o[:, 0], o[:, 1]
        # answer_b = 1^T s_5 = sum(s_4) - <g, s_4>
        tot += s4.sum() - float(g @ s4)
    return np.float32(tot / (NB * H * W))


# revision 9
# speedup vs baseline: 2.7773x; 1.0414x over previous
"""Trainium2 Bass kernel for the CustomCRFLoss problem.

Strategy (pure data parallel, one sample per NeuronCore, 8 cores):

The reference collapses to:
    s_0[j] = colsum(unary)[j],  s_{t+1} = s_t - M s_t,  answer = 1^T s_5
with M[j,w] = M1[j,w] + M2[j,w]:
    M1[j,w] = sum_i k(x_ij, x_iw)   (spatial, row pairs)
    M2[j,w] = sum_i k(x_ij, x_wj)   (bilateral, column pairs)
and k(a,b) = exp(-||a-b||^2/2) = exp(-|a|^2/2) exp(-|b|^2/2) exp(a.b).

Both terms use ONE degree-2 Taylor feature tile TT[p,m,j] = phi_m(x at row
p, col j) (10 monomial features x^alpha/sqrt(alpha!) * exp(-r/2), bf16):
    MT1 = sum_m T_m^T T_m                      (10 PSUM-accumulated matmuls)
    MT2[w,j] = sum_m T_m[w,j] * c_m[j],  c_m[j] = sum_i T_m[i,j]
The partition-broadcast column sums come free from CC = J @ TT with J the
all-ones matrix (3 matmuls), so MT2 is 3 elementwise muls and a small add
tree.  No exps beyond the single E0, no per-column Gram matmuls.

Slot order is chosen so CC chunk 0 = [E0, diag] depends only on E0 (the
earliest features), and the unary runs on the otherwise idle Pool engine.
Tail: 4 iterations of (matvec + subtract); the 5th iteration is folded into
a host-side dot: answer = <1 - rowsum(MT), s_4>, with rowsum(MT) from the
Activation engine's accumulate output.  The kernel ships [s_4 | g] per core.

Emulated accuracy vs the f64 reference: rel err ~1e-3 (gate 2e-2).
"""

import math

import numpy as np

import concourse.bass as bass
import concourse.tile as tile
from concourse import mybir
from concourse.bass_utils import run_bass_kernel_spmd
from concourse.tile import add_dep_helper

H = W = 128
NB = 8  # batch / cores
NMON = 10

F32 = mybir.dt.float32
BF16 = mybir.dt.bfloat16
AF = mybir.ActivationFunctionType
ALU = mybir.AluOpType
AX = mybir.AxisListType

ISQ2 = 1.0 / math.sqrt(2.0)


def _bcast(ap, wid):
    """[P,128] AP -> [P,wid,128] with a step-0 middle dim."""
    return bass.AP(
        tensor=ap.tensor,
        offset=ap.offset,
        ap=[list(ap.ap[0]), [0, wid], list(ap.ap[1])],
    )


def build_kernel():
    nc = bass.Bass()
    im_d = nc.dram_tensor("imb", (H, 3, W), BF16, kind="ExternalInput")
    lg_d = nc.dram_tensor("lg", (H, 2, W), F32, kind="ExternalInput")
    lb_d = nc.dram_tensor("lb", (H, W), F32, kind="ExternalInput")
    out_d = nc.dram_tensor("out", (H, 2), F32, kind="ExternalOutput")

    with tile.TileContext(nc) as tc:
        with (
            tc.tile_pool(name="sb", bufs=1) as sb,
            tc.tile_pool(name="qp", bufs=3) as qpool,
            tc.tile_pool(name="pm", bufs=1, space="PSUM") as pm,
            tc.tile_pool(name="pc", bufs=1, space="PSUM") as pc,
            tc.tile_pool(name="ps", bufs=2, space="PSUM") as ps,
        ):
            # ---------------- input DMAs (dense, partition-major) ----------
            imtile = sb.tile([H, 3, W], BF16)
            nc.sync.dma_start(out=imtile, in_=im_d[:])
            lg = sb.tile([H, 2, W], F32)
            nc.sync.dma_start(out=lg, in_=lg_d[:])
            lb = sb.tile([H, W], F32)
            nc.sync.dma_start(out=lb, in_=lb_d[:])

            # ---------------- constants ----------------
            ones_mat = sb.tile([H, W], BF16)
            nc.gpsimd.memset(ones_mat, 1.0)
            ones_col = sb.tile([H, 1], F32)
            nc.gpsimd.memset(ones_col, 1.0)

            # ------------- feature build (DVE critical path) ---------------
            # xsall = x/sqrt(2); sq2 = x*xsall = x^2/sqrt(2)
            xsall = sb.tile([H, 3, W], BF16)
            nc.vector.tensor_scalar_mul(out=xsall, in0=imtile, scalar1=ISQ2)
            sq2 = sb.tile([H, 3, W], BF16)
            nc.vector.tensor_mul(out=sq2, in0=imtile, in1=xsall)
            # r/sqrt(2) = sum_c sq2_c ; E0 = exp(-r/2) via ACT scale
            a1 = sb.tile([H, W], BF16)
            nc.vector.tensor_add(out=a1, in0=sq2[:, 0, :], in1=sq2[:, 1, :])
            rs = sb.tile([H, W], BF16)
            nc.vector.tensor_add(out=rs, in0=a1, in1=sq2[:, 2, :])

            # TT slots: 0=E0, 1:4=x_c^2/sqrt2*E0 (diag), 4:7=x_c*E0 (deg1),
            # 7=x1x2E0, 8=x1x3E0, 9=x2x3E0
            TT = sb.tile([H, NMON, W], BF16)
            _e0 = nc.scalar.activation(
                out=TT[:, 0, :], in_=rs, func=AF.Exp, scale=-ISQ2
            )
            E0 = TT[:, 0, :]
            nc.vector.tensor_mul(out=TT[:, 1:4, :], in0=sq2, in1=_bcast(E0, 3))
            nc.vector.tensor_mul(out=TT[:, 4:7, :], in0=imtile, in1=_bcast(E0, 3))
            T1 = TT[:, 4, :]
            nc.vector.tensor_mul(
                out=TT[:, 7:9, :], in0=_bcast(T1, 2), in1=imtile[:, 1:3, :]
            )
            nc.vector.tensor_mul(
                out=TT[:, 9:10, :], in0=TT[:, 5:6, :], in1=imtile[:, 2:3, :]
            )

            # ---------------- unary (Pool + ACT, off critical path) --------
            dd = sb.tile([H, W], F32)
            nc.gpsimd.tensor_sub(out=dd, in0=lg[:, 1, :], in1=lg[:, 0, :])
            ed = sb.tile([H, W], F32)
            _ed = nc.scalar.activation(out=ed, in_=dd, func=AF.Exp)
            # keep the ACT queue free for E0 (the global gate) first
            add_dep_helper(_ed.ins, _e0.ins, False, "E0 before unary exp")
            sp = sb.tile([H, W], F32)
            nc.scalar.activation(out=sp, in_=ed, func=AF.Ln, bias=1.0)
            tl = sb.tile([H, W], F32)
            nc.gpsimd.tensor_mul(out=tl, in0=lb, in1=dd)
            u = sb.tile([H, W], F32)
            nc.gpsimd.tensor_sub(out=u, in0=sp, in1=tl)

            # ---------------- PE: CC chunks first, then MT1 ----------------
            mt1 = pm.tile([H, W], F32)
            nc.tensor.matmul(
                mt1, lhsT=TT[:, 0, :], rhs=TT[:, 0, :], start=True, stop=False
            )
            cc0 = pc.tile([H, 4 * W], F32, tag="cc0")
            nc.tensor.matmul(
                cc0, lhsT=ones_mat, rhs=TT[:, 0:4, :], start=True, stop=True
            )
            cc1 = pc.tile([H, 4 * W], F32, tag="cc1")
            nc.tensor.matmul(
                cc1, lhsT=ones_mat, rhs=TT[:, 4:8, :], start=True, stop=True
            )
            cc2 = pc.tile([H, 2 * W], F32, tag="cc2")
            nc.tensor.matmul(
                cc2, lhsT=ones_mat, rhs=TT[:, 8:10, :], start=True, stop=True
            )
            for m in range(1, NMON):
                nc.tensor.matmul(
                    mt1, lhsT=TT[:, m, :], rhs=TT[:, m, :],
                    start=False, stop=(m == NMON - 1),
                )
            q0p = ps.tile([H, 1], F32, tag="s")
            nc.tensor.matmul(q0p, lhsT=u, rhs=ones_col, start=True, stop=True)

            # -------- products (DVE, PSUM-direct) + chunk-local tree -------
            # last chunk goes through an ACT copy so its product is 2x
            ccs2 = sb.tile([H, 2, W], BF16)
            nc.scalar.activation(out=ccs2, in_=cc2, func=AF.Copy)

            P = sb.tile([H, NMON, W], BF16)
            nc.vector.tensor_mul(out=P[:, 0:4, :], in0=TT[:, 0:4, :], in1=cc0)
            # chunk-0 tree branch on the otherwise idle Pool engine
            r1a = sb.tile([H, 2, W], BF16)
            nc.gpsimd.tensor_add(out=r1a, in0=P[:, 0:2, :], in1=P[:, 2:4, :])
            r1s = sb.tile([H, W], BF16)
            nc.gpsimd.tensor_add(out=r1s, in0=r1a[:, 0, :], in1=r1a[:, 1, :])
            nc.vector.tensor_mul(out=P[:, 4:8, :], in0=TT[:, 4:8, :], in1=cc1)
            s1a = sb.tile([H, 2, W], BF16)
            nc.vector.tensor_add(out=s1a, in0=P[:, 4:6, :], in1=P[:, 6:8, :])
            s1s = sb.tile([H, W], BF16)
            nc.vector.tensor_add(out=s1s, in0=s1a[:, 0, :], in1=s1a[:, 1, :])
            nc.vector.tensor_mul(out=P[:, 8:10, :], in0=TT[:, 8:10, :], in1=ccs2)
            t3 = sb.tile([H, W], BF16)
            nc.vector.tensor_add(out=t3, in0=P[:, 8, :], in1=P[:, 9, :])
            f1 = sb.tile([H, W], BF16)
            nc.vector.tensor_add(out=f1, in0=r1s, in1=s1s)
            mt2b = sb.tile([H, W], BF16)
            nc.vector.tensor_add(out=mt2b, in0=f1, in1=t3)

            # ---------------- M, iterations ----------------
            qcur = qpool.tile([H, 1], F32, tag="q")
            nc.vector.tensor_copy(out=qcur, in_=q0p)
            MTs = sb.tile([H, W], F32)
            nc.vector.tensor_add(out=MTs, in0=mt1, in1=mt2b)

            # out layout: vout[:,0]=s_4, vout[:,1]=g=rowsum(MT)
            vout = sb.tile([H, 2], F32)
            gdump = sb.tile([H, W], BF16)
            nc.scalar.activation(
                out=gdump, in_=MTs, func=AF.Copy, accum_out=vout[:, 1:2]
            )

            for it in range(4):
                yp = ps.tile([H, 1], F32, tag="s")
                nc.tensor.matmul(yp, lhsT=MTs, rhs=qcur, start=True, stop=True)
                if it < 3:
                    qn = qpool.tile([H, 1], F32, tag="q")
                    nc.vector.tensor_sub(out=qn, in0=qcur, in1=yp)
                    qcur = qn
                else:
                    nc.vector.tensor_sub(out=vout[:, 0:1], in0=qcur, in1=yp)

            nc.sync.dma_start(out=out_d[:], in_=vout)

    return nc


def _split_excess_waits(nc, max_waits=1, max_updates=1):
    """The walrus build in this container rejects instructions whose Events
    carry more than one semaphore wait (ISA Events has a single wait slot).
    Tile's sem assignment can attach several.  Split the extras onto
    same-engine NoOps placed immediately before (waits) / after (updates)
    the instruction; sequencers execute in order, so semantics are kept."""
    for fn in nc.m.functions:
        for bb in fn.blocks:
            ins = bb.instructions
            out = []
            changed = False
            for inst in ins:
                si = inst.sync_info
                if si is None:
                    out.append(inst)
                    continue
                waits = list(si.on_wait or [])
                updates = list(si.on_update or [])
                if len(waits) <= max_waits and len(updates) <= max_updates:
                    out.append(inst)
                    continue
                changed = True
                pre, post = [], []
                if len(waits) > max_waits:
                    for k, wt in enumerate(waits[:-max_waits]):
                        pre.append(
                            mybir.InstNoOp(
                                name=f"{inst.name}-w{k}",
                                engine=inst.engine,
                                bass_nofuse=True,
                                sync_info=mybir.SyncInfo(on_wait=[wt], on_update=[]),
                            )
                        )
                    waits = waits[-max_waits:]
                if len(updates) > max_updates:
                    for k, up in enumerate(updates[max_updates:]):
                        post.append(
                            mybir.InstNoOp(
                                name=f"{inst.name}-u{k}",
                                engine=inst.engine,
                                bass_nofuse=True,
                                sync_info=mybir.SyncInfo(on_wait=[], on_update=[up]),
                            )
                        )
                    updates = updates[:max_updates]
                inst.sync_info = mybir.SyncInfo(on_wait=waits, on_update=updates)
                out.extend(pre)
                out.append(inst)
                out.extend(post)
            if changed:
                bb.instructions = out
    return nc


_NC_CACHE = None


def kernel(logits, labels, images):
    global _NC_CACHE
    if _NC_CACHE is None:
        _NC_CACHE = _split_excess_waits(build_kernel())
    nc = _NC_CACHE

    import ml_dtypes

    logits = np.asarray(logits, dtype=np.float32)
    labels_f = np.asarray(labels).astype(np.float32)
    images = np.asarray(images, dtype=np.float32)
    imc = images - 0.5
    # partition-major layouts: [H, C, W]
    im_b = np.ascontiguousarray(np.swapaxes(imc, 1, 2).astype(ml_dtypes.bfloat16))
    lg_t = np.ascontiguousarray(np.swapaxes(logits, 1, 2))

    in_maps = [
        {
            "imb": im_b[b],
            "lg": lg_t[b],
            "lb": np.ascontiguousarray(labels_f[b]),
        }
        for b in range(NB)
    ]
    res = run_bass_kernel_spmd(nc, in_maps, core_ids=list(range(NB)))
    tot = 0.0
    for b in range(NB):
        o = res.results[b]["out"].astype(np.float64)
        s4, g = o[:, 0], o[:, 1]
        # answer_b = 1^T s_5 = sum(s_4) - <g, s_4>
        tot += s4.sum() - float(g @ s4)
    return np.float32(tot / (NB * H * W))
